# revision 1
# baseline (speedup 1.0000x reference)
"""Local (sliding-window) self-attention Bass kernel for 8 TRN2 NeuronCores.

Problem: B=4, T=4096, C=512, H=8 heads, head_dim=64, window=15.
Sharding: 8 cores = batch(4) x seq-halves(2). Each core processes 2048 query
tokens of one batch element; its x chunk carries a 7-token halo on each side
(zero-padded at sequence edges, matching the reference's jnp.pad semantics),
padded to 2080 rows for 128/32 alignment.

Per-core dataflow (bf16 matmuls, fp32 PSUM accumulation):
  x chunk --mask*cast--> x_tok bf16 --XBAR DMA transpose--> xT (feature-major)
  qT = Wq-stationary matmuls + bias (feature-major, scores lhsT)
  kT = Wkv[:, :C]-stationary matmuls + bias (feature-major, scores rhs)
  v_tok = xT-stationary matmuls + bias (token-major, AV rhs-source)
  per 128-token block x head-pair:
    scores [128q, 160k] matmul -> exp(scale*s) on ACT -> band-mask*accum on DVE
    -> normalize rows -> XBAR DMA transpose -> alphaT
    attnT [2*64d, 128q] = v.T @ alphaT (two matmuls, K=128 + K=32)
  out = attnT-stationary proj matmuls + bproj + mask -> DMA (token-major)
"""

import math
import os
from contextlib import ExitStack

import ml_dtypes
import numpy as np

import concourse.bacc as bacc
import concourse.bass as bass
import concourse.mybir as mybir
import concourse.tile as tile
from concourse import bass_utils

B, T, C, H, WIN = 4, 4096, 512, 8, 15
D = C // H            # 64
PAD = WIN // 2        # 7
NTOK = T // 2         # 2048 query tokens per core
NKV = 2080            # kv rows per core: 7 + 2048 + 7 = 2062, padded to 2080
NB = NTOK // 128      # 16 query blocks
KCH = [512, 512, 512, 512, 32]  # kv token chunks for feature-major matmuls
SCALE = math.log(WIN) / D
F32 = mybir.dt.float32
BF16 = mybir.dt.bfloat16


def _band_mask() -> np.ndarray:
    """[128,160] multiplicative band: band[p, j] = 1 iff p <= j <= p+14."""
    p = np.arange(128)[:, None]
    j = np.arange(160)[None, :]
    return ((j >= p) & (j <= p + WIN - 1)).astype(ml_dtypes.bfloat16)


def build_program() -> bacc.Bacc:
    nc = bacc.Bacc("TRN2", target_bir_lowering=False, debug=False,
                   enable_asserts=False, num_devices=8)

    xd = nc.dram_tensor("x", [NKV, C], F32, kind="ExternalInput").ap()
    maskd = nc.dram_tensor("mask", [NKV], F32, kind="ExternalInput").ap()
    wqd = nc.dram_tensor("wq", [C, C], F32, kind="ExternalInput").ap()
    bqd = nc.dram_tensor("bq", [C], F32, kind="ExternalInput").ap()
    wkvd = nc.dram_tensor("wkv", [C, 2 * C], F32, kind="ExternalInput").ap()
    bkvd = nc.dram_tensor("bkv", [2 * C], F32, kind="ExternalInput").ap()
    wpd = nc.dram_tensor("wproj", [C, C], F32, kind="ExternalInput").ap()
    bpd = nc.dram_tensor("bproj", [C], F32, kind="ExternalInput").ap()
    bandd = nc.dram_tensor("band", [128, 160], BF16, kind="ExternalInput").ap()
    outd = nc.dram_tensor("out", [NTOK, C], F32, kind="ExternalOutput").ap()

    with tile.TileContext(nc) as tc, ExitStack() as ctx:
        sb = ctx.enter_context(tc.tile_pool(name="sb", bufs=1))
        sb_x = ctx.enter_context(tc.tile_pool(name="sb_x", bufs=3))
        sb_a = ctx.enter_context(tc.tile_pool(name="sb_a", bufs=3))
        sb_o = ctx.enter_context(tc.tile_pool(name="sb_o", bufs=3))
        pp_big = ctx.enter_context(tc.tile_pool(name="pp_big", bufs=2, space="PSUM"))
        pp_sc = ctx.enter_context(tc.tile_pool(name="pp_sc", bufs=4, space="PSUM"))
        pp_at = ctx.enter_context(tc.tile_pool(name="pp_at", bufs=2, space="PSUM"))

        # ---- persistent SBUF tensors ----
        xT = [sb.tile([128, NKV], BF16, tag=f"xT{i}", name=f"xT{i}") for i in range(4)]
        qT = [sb.tile([128, NTOK], BF16, tag=f"qT{i}", name=f"qT{i}") for i in range(4)]
        kT = [sb.tile([128, NKV], BF16, tag=f"kT{i}", name=f"kT{i}") for i in range(4)]
        v_tok = [sb.tile([128, C], BF16, tag=f"vtok{i}", name=f"vtok{i}") for i in range(17)]
        aT = [sb.tile([128, NTOK], BF16, tag=f"aT{i}", name=f"aTt{i}") for i in range(4)]
        band = sb.tile([128, 160], BF16, tag="band")
        wq = [sb.tile([128, C], BF16, tag=f"wq{i}", name=f"wq{i}") for i in range(4)]
        wk = [sb.tile([128, C], BF16, tag=f"wk{i}", name=f"wk{i}") for i in range(4)]
        wv = [sb.tile([128, C], BF16, tag=f"wv{i}", name=f"wv{i}") for i in range(4)]
        wp = [sb.tile([128, C], BF16, tag=f"wp{i}", name=f"wp{i}") for i in range(4)]
        bq_t = sb.tile([128, 4], F32, tag="bq")       # per-partition q bias
        bk_t = sb.tile([128, 4], F32, tag="bk")       # per-partition k bias
        bvB = sb.tile([128, C], F32, tag="bvB")       # v bias bcast over partitions
        bpB = sb.tile([128, C], F32, tag="bpB")       # proj bias bcast
        mq = sb.tile([128, NB], F32, tag="mq")        # query-token mask, per block
        al_bufs = [sb.tile([128, 448], BF16, tag=f"al{j}", name=f"al{j}")
                   for j in range(3)]
        for j in range(3):
            nc.gpsimd.memset(al_bufs[j][:, 320:448], 0.0)

        # ---- constants / weights in ----
        _REP = int(os.environ.get("KREP", "1"))
        for _rep in range(_REP):
         nc.sync.dma_start(band[:], bandd)
         nc.sync.dma_start(bq_t[:], bqd.rearrange("(a b) -> b a", b=128))
         nc.sync.dma_start(bk_t[:], bkvd[0:C].rearrange("(a b) -> b a", b=128))
         nc.sync.dma_start(bvB[:], bkvd[C:2 * C][None, :].broadcast_to((128, C)))
         nc.sync.dma_start(bpB[:], bpd[None, :].broadcast_to((128, C)))
         nc.sync.dma_start(mq[:], maskd[PAD:PAD + NTOK].rearrange("(a b) -> b a", b=128))
         for ci in range(4):
             wqf = sb_x.tile([128, C], F32, tag="wld")
             nc.sync.dma_start(wqf[:], wqd[ci * 128:(ci + 1) * 128, :])
             nc.vector.tensor_copy(wq[ci][:], wqf[:])
             wkf = sb_x.tile([128, 2 * C], F32, tag="wld2")
             nc.sync.dma_start(wkf[:], wkvd[ci * 128:(ci + 1) * 128, :])
             nc.vector.tensor_copy(wk[ci][:], wkf[:, 0:C])
             nc.vector.tensor_copy(wv[ci][:], wkf[:, C:2 * C])
             wpf = sb_x.tile([128, C], F32, tag="wld")
             nc.sync.dma_start(wpf[:], wpd[ci * 128:(ci + 1) * 128, :])
             nc.vector.tensor_copy(wp[ci][:], wpf[:])

         # ---- x in: mask*cast, then XBAR-transpose to feature-major ----
         for t in range(17):
             r0, r1 = t * 128, min((t + 1) * 128, NKV)
             rows = r1 - r0
             xf = sb_x.tile([128, C], F32, tag="xf")
             nc.sync.dma_start(xf[:rows, :], xd[r0:r1, :])
             mrow = sb_x.tile([128, 1], F32, tag="mrow")
             nc.sync.dma_start(mrow[:rows, :], maskd[r0:r1][:, None])
             xb = sb_x.tile([128, C], BF16, tag="xb")
             nc.vector.tensor_scalar_mul(xb[:rows, :], xf[:rows, :], mrow[:rows, :])
             for ci in range(4):
                 nc.scalar.dma_start_transpose(
                     xT[ci][:, r0:r1], xb[:rows, ci * 128:(ci + 1) * 128])

         # ---- qT (feature-major): W stationary, xT moving ----
         for co in range(4):
             for ch in range(4):
                 t0 = ch * 512
                 ps = pp_big.tile([128, 512], F32, tag="big")
                 for ci in range(4):
                     nc.tensor.matmul(
                         ps[:], wq[ci][:, co * 128:(co + 1) * 128],
                         xT[ci][:, PAD + t0:PAD + t0 + 512],
                         start=(ci == 0), stop=(ci == 3))
                 nc.scalar.activation(qT[co][:, t0:t0 + 512], ps[:],
                                      mybir.ActivationFunctionType.Identity,
                                      bias=bq_t[:, co:co + 1])

         # ---- kT (feature-major) ----
         for co in range(4):
             t0 = 0
             for w in KCH:
                 ps = pp_big.tile([128, 512], F32, tag="big")
                 for ci in range(4):
                     nc.tensor.matmul(
                         ps[:, 0:w], wk[ci][:, co * 128:(co + 1) * 128],
                         xT[ci][:, t0:t0 + w],
                         start=(ci == 0), stop=(ci == 3))
                 nc.scalar.activation(kT[co][:, t0:t0 + w], ps[:, 0:w],
                                      mybir.ActivationFunctionType.Identity,
                                      bias=bk_t[:, co:co + 1])
                 t0 += w

         # ---- v_tok (token-major): xT stationary, Wv moving ----
         for t in range(17):
             r0, r1 = t * 128, min((t + 1) * 128, NKV)
             rows = r1 - r0
             ps = pp_big.tile([128, 512], F32, tag="big")
             for ci in range(4):
                 nc.tensor.matmul(
                     ps[:rows, :], xT[ci][:, r0:r1],
                     wv[ci][:], start=(ci == 0), stop=(ci == 3))
             nc.vector.scalar_tensor_tensor(
                 v_tok[t][:rows, :], ps[:rows, :], 1.0, bvB[:rows, :],
                 op0=mybir.AluOpType.mult, op1=mybir.AluOpType.add)

         # ---- attention: per 128-token block, heads in pairs ----
         _PH = int(os.environ.get("KPHASE", "3"))
         if _PH < 2:
             dbg = sb.tile([128, C], F32, tag="dbg")
             nc.vector.tensor_copy(dbg[:], v_tok[0][:])
             nc.sync.dma_start(outd[0:128, :], dbg[:])
         _KS = int(os.environ.get("KSUB", "4"))
         for i in range(min(NB, int(os.environ.get("KNB", str(NB)))) if _PH >= 2 else 0):
             for hp in range(4):                      # head pair -> c-tile hp
                 al = al_bufs[(i * 4 + hp) % 3]
                 for hh in range(2):                  # head h = 2*hp + hh
                     sc = pp_sc.tile([128, 160], F32, tag="sc")
                     nc.tensor.matmul(
                         sc[:],
                         qT[hp][hh * 64:(hh + 1) * 64, i * 128:(i + 1) * 128],
                         kT[hp][hh * 64:(hh + 1) * 64, i * 128:i * 128 + 160],
                         start=True, stop=True)
                     nc.scalar.activation(al[:, hh * 160:hh * 160 + 160], sc[:],
                                          mybir.ActivationFunctionType.Exp,
                                          scale=SCALE)
                 at_ps = pp_at.tile([128, 128], F32, tag="at")
                 for hh in range(2):
                     a = al[:, hh * 160:hh * 160 + 160]
                     if _KS >= 2:
                         den = sb_a.tile([128, 1], F32, tag="den")
                         nc.vector.scalar_tensor_tensor(
                             a, a, 1.0, band[:],
                             op0=mybir.AluOpType.mult, op1=mybir.AluOpType.mult,
                             accum_out=den[:])
                         rden = sb_a.tile([128, 1], F32, tag="rden")
                         nc.vector.reciprocal(rden[:], den[:])
                         nc.vector.tensor_scalar_mul(a, a, rden[:])
                     if _KS >= 3:
                         aT1 = sb_a.tile([128, 128], BF16, tag="aT1")
                         aT2 = sb_a.tile([128, 128], BF16, tag="aT2")
                         c0 = hh * 160
                         nc.scalar.dma_start_transpose(aT1[:], al[:, c0:c0 + 128])
                         nc.scalar.dma_start_transpose(aT2[:], al[:, c0 + 128:c0 + 256])
                     if _KS >= 4:
                         nc.tensor.matmul(
                             at_ps[hh * 64:(hh + 1) * 64, :],
                             v_tok[i][:, hp * 128 + hh * 64:hp * 128 + (hh + 1) * 64],
                             aT1[:], start=True, stop=False)
                         nc.tensor.matmul(
                             at_ps[hh * 64:(hh + 1) * 64, :],
                             v_tok[i + 1][0:32, hp * 128 + hh * 64:hp * 128 + (hh + 1) * 64],
                             aT2[0:32, :], start=False, stop=True)
                 if _KS >= 4:
                     nc.scalar.activation(aT[hp][:, i * 128:(i + 1) * 128], at_ps[:],
                                          mybir.ActivationFunctionType.Copy)

         # ---- proj (token-major): attnT stationary, Wproj moving ----
         if _PH == 2:
             dbg = sb.tile([128, 320], F32, tag="dbg")
             nc.vector.tensor_copy(dbg[:], aT[0][:, 0:320] if _KS >= 4 else al_bufs[0][:, 0:320])
             nc.sync.dma_start(outd[0:128, 0:320], dbg[:])
         for t in range(NB if _PH >= 3 else 0):
             ps = pp_big.tile([128, 512], F32, tag="big")
             for ci in range(4):
                 nc.tensor.matmul(
                     ps[:], aT[ci][:, t * 128:(t + 1) * 128],
                     wp[ci][:], start=(ci == 0), stop=(ci == 3))
             bm = sb_o.tile([128, C], F32, tag="bm")
             nc.gpsimd.tensor_scalar_mul(bm[:], bpB[:], mq[:, t:t + 1])
             ot = sb_o.tile([128, C], F32, tag="ot")
             nc.vector.scalar_tensor_tensor(
                 ot[:], ps[:], mq[:, t:t + 1], bm[:],
                 op0=mybir.AluOpType.mult, op1=mybir.AluOpType.add)
             nc.sync.dma_start(outd[t * 128:(t + 1) * 128, :], ot[:])

    nc.compile()
    return nc


_CACHE: dict = {}


def _get_program() -> bacc.Bacc:
    if "nc" not in _CACHE:
        _CACHE["nc"] = build_program()
    return _CACHE["nc"]


def kernel(x, mask, Wq, bq, Wkv, bkv, Wproj, bproj) -> np.ndarray:
    x = np.asarray(x, np.float32)
    mask = np.asarray(mask, np.float32)
    band = np.ascontiguousarray(_band_mask())
    nc = _get_program()

    in_maps = []
    for core in range(8):
        b, h = divmod(core, 2)
        s = h * NTOK
        xc = np.zeros((NKV, C), np.float32)
        mc = np.zeros((NKV,), np.float32)
        lo, hi = max(0, s - PAD), min(T, s + NTOK + PAD)
        xc[lo - (s - PAD):lo - (s - PAD) + hi - lo] = x[b, lo:hi]
        mc[lo - (s - PAD):lo - (s - PAD) + hi - lo] = mask[b, lo:hi]
        in_maps.append({
            "x": xc, "mask": mc,
            "wq": np.asarray(Wq, np.float32), "bq": np.asarray(bq, np.float32),
            "wkv": np.asarray(Wkv, np.float32), "bkv": np.asarray(bkv, np.float32),
            "wproj": np.asarray(Wproj, np.float32),
            "bproj": np.asarray(bproj, np.float32),
            "band": band,
        })

    res = bass_utils.run_bass_kernel_spmd(nc, in_maps, core_ids=list(range(8)))
    out = np.empty((B, T, C), np.float32)
    for core in range(8):
        b, h = divmod(core, 2)
        out[b, h * NTOK:(h + 1) * NTOK] = res.results[core]["out"]
    return out



# revision 11
# speedup vs baseline: 2.6304x; 2.6304x over previous
"""Local (sliding-window) self-attention Bass kernel for 8 TRN2 NeuronCores.

Problem: B=4, T=4096, C=512, H=8 heads, head_dim=64, window=15.
Sharding: 8 cores = batch(4) x seq-halves(2). Each core processes 2048 query
tokens of one batch element; its x chunk carries a 7-token halo on each side
(zero-padded at sequence edges, matching the reference's jnp.pad semantics),
padded to 2080 rows for 128/32 alignment.

v2 dataflow (no DMA transposes; scores computed k-major so alpha never needs
transposing; per-128-token-block batching of ACT/DVE work):
  x chunk --mask*cast--> x_tok bf16 --PE transpose--> xT (feature-major)
  qT/kT = W-stationary matmuls + bias (feature-major)
  v_tok = xT-stationary matmuls + bias, token-major, packed [64 v | 1 ones]
          per head (the ones column makes AV emit the softmax denominator)
  per 128-query block:
    scoresT [k,q] = kT-slice.T @ qT-slice per head (A:128k + B:16k chunks)
    one Exp over the whole block's scores -> alphaT bf16
    one DVE band-mask multiply
    AV token-major: av[q, 65] per head = alphaT.T @ v_aug  (den in col 64)
    reciprocal + one DVE normalize (rden broadcast via 0-stride AP) -> bf16
    4 PE transposes -> attnT feature-major -> one ACT copy -> aT
    proj: aT-stationary matmuls + bias + mask -> DMA out (token-major)
"""

import math
import os
from contextlib import ExitStack

import ml_dtypes
import numpy as np

import concourse.bacc as bacc
import concourse.bass as bass
import concourse.mybir as mybir
import concourse.tile as tile
from concourse import bass_utils

B, T, C, H, WIN = 4, 4096, 512, 8, 15
D = C // H            # 64
PAD = WIN // 2        # 7
NTOK = T // 2         # 2048 query tokens per core
NKV = 2112            # kv rows per core: 7 + 2048 + 7 = 2062, padded to 2112
                      # (the per-block k-tail chunk is 64 wide: 16*128+64)
NB = NTOK // 128      # 16 query blocks
SCALE = math.log(WIN) / D
F32 = mybir.dt.float32
BF16 = mybir.dt.bfloat16


def _band_mask() -> np.ndarray:
    """[128, 2048] multiplicative band, k-major (scoresT layout).

    Cols h*128+q (h=0..7): A-chunk, row k (0..127): 1 iff k-14 <= q <= k.
    Cols 1024+h*128+q: B-chunk (k-tail), row r (0..63): 1 iff q >= 114+r
    (rows 64..127 of the B region are unused and left zero).
    """
    out = np.zeros((128, 2048), np.float32)
    k = np.arange(128)[:, None]
    q = np.arange(128)[None, :]
    a = ((q >= k - 14) & (q <= k)).astype(np.float32)
    r = np.arange(64)[:, None]
    b = (q >= 114 + r).astype(np.float32)
    for h in range(8):
        out[:, h * 128:(h + 1) * 128] = a
        out[0:64, 1024 + h * 128:1024 + (h + 1) * 128] = b
    return out.astype(ml_dtypes.bfloat16)


def _identity() -> np.ndarray:
    return np.eye(128, dtype=ml_dtypes.bfloat16)


def build_program() -> bacc.Bacc:
    nc = bacc.Bacc("TRN2", target_bir_lowering=False, debug=False,
                   enable_asserts=False, num_devices=8)

    xd = nc.dram_tensor("x", [NKV, C], F32, kind="ExternalInput").ap()
    maskd = nc.dram_tensor("mask", [NKV], F32, kind="ExternalInput").ap()
    wqd = nc.dram_tensor("wq", [C, C], F32, kind="ExternalInput").ap()
    bqd = nc.dram_tensor("bq", [C], F32, kind="ExternalInput").ap()
    wkvd = nc.dram_tensor("wkv", [C, 2 * C], F32, kind="ExternalInput").ap()
    bkvd = nc.dram_tensor("bkv", [2 * C], F32, kind="ExternalInput").ap()
    wpd = nc.dram_tensor("wproj", [C, C], F32, kind="ExternalInput").ap()
    bpd = nc.dram_tensor("bproj", [C], F32, kind="ExternalInput").ap()
    bandd = nc.dram_tensor("band", [128, 2048], BF16, kind="ExternalInput").ap()
    identd = nc.dram_tensor("ident", [128, 128], BF16, kind="ExternalInput").ap()
    outd = nc.dram_tensor("out", [NTOK, C], F32, kind="ExternalOutput").ap()

    with tile.TileContext(nc) as tc, ExitStack() as ctx:
        sb = ctx.enter_context(tc.tile_pool(name="sb", bufs=1))
        sb_ld = ctx.enter_context(tc.tile_pool(name="sb_ld", bufs=3))
        sb_a = ctx.enter_context(tc.tile_pool(name="sb_a", bufs=3))
        sb_o = ctx.enter_context(tc.tile_pool(name="sb_o", bufs=3))
        pp_sc = ctx.enter_context(tc.tile_pool(name="pp_sc", bufs=1, space="PSUM"))
        pp_tr = ctx.enter_context(tc.tile_pool(name="pp_tr", bufs=1, space="PSUM"))
        pp_pr = ctx.enter_context(tc.tile_pool(name="pp_pr", bufs=1, space="PSUM"))
        pp_av = ctx.enter_context(tc.tile_pool(name="pp_av", bufs=1, space="PSUM"))

        # ---- persistent SBUF tensors ----
        xT = sb.tile([128, 4 * NKV], BF16, tag="xT")     # col ci*NKV + t
        qT = sb.tile([128, 4 * NTOK], BF16, tag="qT")    # col co*NTOK + t
        kT = sb.tile([128, 4 * NKV], BF16, tag="kT")     # col co*NKV + t
        aT = sb.tile([128, 4 * NTOK], BF16, tag="aT")    # col ct*NTOK + q
        v_tok = [sb.tile([128, 520], BF16, tag=f"vtok{i}", name=f"vtok{i}")
                 for i in range(17)]                     # col h*65: [64 v | 1]
        band = sb.tile([128, 2048], BF16, tag="band")
        ident = sb.tile([128, 128], BF16, tag="ident")
        wq = [sb.tile([128, C], BF16, tag=f"wq{i}", name=f"wq{i}") for i in range(4)]
        wk = [sb.tile([128, C], BF16, tag=f"wk{i}", name=f"wk{i}") for i in range(4)]
        wv = [sb.tile([128, C], BF16, tag=f"wv{i}", name=f"wv{i}") for i in range(4)]
        wp = [sb.tile([128, C], BF16, tag=f"wp{i}", name=f"wp{i}") for i in range(4)]
        bq_t = sb.tile([128, 4], F32, tag="bq")       # per-partition q bias
        bk_t = sb.tile([128, 4], F32, tag="bk")       # per-partition k bias
        bvB = sb.tile([128, C], F32, tag="bvB")       # v bias bcast over partitions
        bpB = sb.tile([128, C], F32, tag="bpB")       # proj bias bcast
        mq = sb.tile([128, NB], F32, tag="mq")        # query-token mask, per block

        # ---- constants / weights in ----
        nc.sync.dma_start(band[:], bandd)
        nc.sync.dma_start(ident[:], identd)
        nc.sync.dma_start(bq_t[:], bqd.rearrange("(a b) -> b a", b=128))
        nc.sync.dma_start(bk_t[:], bkvd[0:C].rearrange("(a b) -> b a", b=128))
        nc.sync.dma_start(bvB[:], bkvd[C:2 * C][None, :].broadcast_to((128, C)))
        nc.sync.dma_start(bpB[:], bpd[None, :].broadcast_to((128, C)))
        nc.sync.dma_start(mq[:], maskd[PAD:PAD + NTOK].rearrange("(a b) -> b a", b=128))
        for ci in range(4):
            wqf = sb_ld.tile([128, C], F32, tag="wld")
            nc.sync.dma_start(wqf[:], wqd[ci * 128:(ci + 1) * 128, :])
            nc.vector.tensor_copy(wq[ci][:], wqf[:])
            wkf = sb_ld.tile([128, 2 * C], F32, tag="wld2")
            nc.sync.dma_start(wkf[:], wkvd[ci * 128:(ci + 1) * 128, :])
            nc.vector.tensor_copy(wk[ci][:], wkf[:, 0:C])
            nc.vector.tensor_copy(wv[ci][:], wkf[:, C:2 * C])
            wpf = sb_ld.tile([128, C], F32, tag="wld")
            nc.sync.dma_start(wpf[:], wpd[ci * 128:(ci + 1) * 128, :])
            nc.vector.tensor_copy(wp[ci][:], wpf[:])

        # ---- x in: mask*cast, then PE-transpose to feature-major ----
        for t in range(17):
            r0, r1 = t * 128, min((t + 1) * 128, NKV)
            rows = r1 - r0
            xf = sb_ld.tile([128, C], F32, tag="xf")
            nc.sync.dma_start(xf[:rows, :], xd[r0:r1, :])
            mrow = sb_ld.tile([128, 1], F32, tag="mrow")
            nc.sync.dma_start(mrow[:rows, :], maskd[r0:r1][:, None])
            xb = sb_ld.tile([128, C], BF16, tag="xb")
            nc.vector.tensor_scalar_mul(xb[:rows, :], xf[:rows, :], mrow[:rows, :])
            xtr = pp_tr.tile([128, 512], BF16, tag="tr", name="xtr")
            for ci in range(4):
                nc.tensor.transpose(
                    xtr[:, ci * 128:ci * 128 + rows],
                    xb[:rows, ci * 128:(ci + 1) * 128],
                    ident[:rows, :rows])
            nc.scalar.activation(
                xT.rearrange("p (a c) -> p a c", a=4)[:, :, r0:r1],
                xtr.rearrange("p (a c) -> p a c", a=4)[:, :, 0:rows],
                mybir.ActivationFunctionType.Copy)

        # Alternate projection-phase PSUM tiles between the two big pools so
        # consecutive chunks double-buffer (each pool alone has bufs=1).
        pcnt = [0]

        def proj_ps():
            pool = pp_sc if pcnt[0] % 2 == 0 else pp_av
            pcnt[0] += 1
            shape = [128, 2048] if pool is pp_sc else [128, 1024]
            return pool.tile(shape, F32, tag="sc" if pool is pp_sc else "av",
                             name=f"pps{pcnt[0]}")

        # ---- qT (feature-major): W stationary, xT moving ----
        _P1 = 4 if int(os.environ.get("KPH", "5")) >= 1 else 0
        for co in range(_P1):
            for ch in range(4):
                t0 = ch * 512
                ps = proj_ps()
                for ci in range(4):
                    nc.tensor.matmul(
                        ps[:, 0:512], wq[ci][:, co * 128:(co + 1) * 128],
                        xT[:, ci * NKV + PAD + t0:ci * NKV + PAD + t0 + 512],
                        start=(ci == 0), stop=(ci == 3))
                nc.scalar.activation(qT[:, co * NTOK + t0:co * NTOK + t0 + 512],
                                     ps[:, 0:512],
                                     mybir.ActivationFunctionType.Identity,
                                     bias=bq_t[:, co:co + 1])

        # ---- kT (feature-major) ----
        KCH = [512, 512, 512, 512, 64]
        for co in range(_P1):
            t0 = 0
            for w in KCH:
                ps = proj_ps()
                for ci in range(4):
                    nc.tensor.matmul(
                        ps[:, 0:w], wk[ci][:, co * 128:(co + 1) * 128],
                        xT[:, ci * NKV + t0:ci * NKV + t0 + w],
                        start=(ci == 0), stop=(ci == 3))
                nc.scalar.activation(kT[:, co * NKV + t0:co * NKV + t0 + w],
                                     ps[:, 0:w],
                                     mybir.ActivationFunctionType.Identity,
                                     bias=bk_t[:, co:co + 1])
                t0 += w

        # ---- v_tok (token-major, packed [64 v | ones] per head) ----
        for t in range(17 if _P1 else 0):
            r0, r1 = t * 128, min((t + 1) * 128, NKV)
            rows = r1 - r0
            ps = proj_ps()
            for ci in range(4):
                nc.tensor.matmul(
                    ps[:rows, 0:512], xT[:, ci * NKV + r0:ci * NKV + r1],
                    wv[ci][:], start=(ci == 0), stop=(ci == 3))
            vv = v_tok[t].rearrange("p (h y) -> p h y", h=8)
            nc.gpsimd.memset(vv[:, :, 64:65], 1.0)
            nc.vector.scalar_tensor_tensor(
                vv[:rows, :, 0:64],
                ps[:rows, 0:512].rearrange("p (h y) -> p h y", h=8),
                1.0,
                bvB.rearrange("p (h y) -> p h y", h=8)[:rows],
                op0=mybir.AluOpType.mult, op1=mybir.AluOpType.add)

        # ---- attention: per 128-query block ----
        KPH = int(os.environ.get("KPH", "5"))
        for i in range(NB if KPH >= 2 else 0):
            sc = pp_sc.tile([128, 2048], F32, tag="sc")
            # heads grouped by operand partition base per PSUM bank:
            # a matmul's tile_position row (= lhsT/rhs base partition) must be
            # uniform within a bank, so even heads (base 0) fill bank 0 and
            # odd heads (base 64) fill bank 1.
            for h in range(8):
                co, hr = h // 2, (h % 2) * 64
                ac = (h % 2) * 512 + (h // 2) * 128
                nc.tensor.matmul(
                    sc[:, ac:ac + 128],
                    kT[hr:hr + 64, co * NKV + i * 128:co * NKV + i * 128 + 128],
                    qT[hr:hr + 64, co * NTOK + i * 128:co * NTOK + (i + 1) * 128],
                    start=True, stop=True)
            for h in range(8 if int(os.environ.get("KSC", "3")) >= 2 else 0):
                co, hr = h // 2, (h % 2) * 64
                bc = 1024 + (h % 2) * 512 + (h // 2) * 128
                nc.tensor.matmul(
                    sc[0:64, bc:bc + 128],
                    kT[hr:hr + 64, co * NKV + i * 128 + 128:co * NKV + i * 128 + 192],
                    qT[hr:hr + 64, co * NTOK + i * 128:co * NTOK + (i + 1) * 128],
                    start=True, stop=True)
            alpha = sb_a.tile([128, 2048], BF16, tag="alpha")
            if int(os.environ.get("KSC", "3")) < 3:
                continue
            nc.scalar.activation(alpha[:, 0:512], sc[:, 0:512],
                                 mybir.ActivationFunctionType.Exp, scale=SCALE)
            nc.scalar.activation(alpha[:, 512:1024], sc[:, 512:1024],
                                 mybir.ActivationFunctionType.Exp, scale=SCALE)
            nc.scalar.activation(alpha[0:64, 1024:1536], sc[0:64, 1024:1536],
                                 mybir.ActivationFunctionType.Exp, scale=SCALE)
            nc.scalar.activation(alpha[0:64, 1536:2048], sc[0:64, 1536:2048],
                                 mybir.ActivationFunctionType.Exp, scale=SCALE)
            if KPH < 3:
                continue
            alm = sb_a.tile([128, 2048], BF16, tag="alm")
            nc.vector.scalar_tensor_tensor(
                alm[:, 0:1024], alpha[:, 0:1024], 1.0, band[:, 0:1024],
                op0=mybir.AluOpType.mult, op1=mybir.AluOpType.mult)
            nc.vector.scalar_tensor_tensor(
                alm[0:64, 1024:2048], alpha[0:64, 1024:2048], 1.0,
                band[0:64, 1024:2048],
                op0=mybir.AluOpType.mult, op1=mybir.AluOpType.mult)

            av = pp_av.tile([128, 1024], F32, tag="av")
            for h in range(8):
                c0 = (h // 4) * 512 + (h % 4) * 65
                ac = (h % 2) * 512 + (h // 2) * 128
                bc = 1024 + (h % 2) * 512 + (h // 2) * 128
                nc.tensor.matmul(
                    av[:, c0:c0 + 65],
                    alm[:, ac:ac + 128],
                    v_tok[i][:, h * 65:h * 65 + 65],
                    start=True, stop=False)
                nc.tensor.matmul(
                    av[:, c0:c0 + 65],
                    alm[0:64, bc:bc + 128],
                    v_tok[i + 1][0:64, h * 65:h * 65 + 65],
                    start=False, stop=True)
            avv = (av.rearrange("p (a c) -> p a c", a=2)[:, :, 0:260]
                     .rearrange("p a (h y) -> p a h y", h=4))
            rden = sb_o.tile([128, 8], F32, tag="rden")
            nc.vector.reciprocal(rden.rearrange("p (a h) -> p a h", a=2),
                                 avv[:, :, :, 64:65].squeeze(3))
            avn = sb_o.tile([128, 512], BF16, tag="avn")
            for a in range(2):
                nc.vector.scalar_tensor_tensor(
                    avn[:, a * 256:(a + 1) * 256]
                       .rearrange("p (h y) -> p h y", h=4),
                    avv[:, a:a + 1, :, 0:64].squeeze(1), 1.0,
                    rden[:, a * 4:(a + 1) * 4].unsqueeze(2)
                        .broadcast_to((128, 4, 64)),
                    op0=mybir.AluOpType.mult, op1=mybir.AluOpType.mult)

            if KPH < 4:
                continue
            tr = pp_tr.tile([128, 512], BF16, tag="tr")
            for ct in range(4):
                nc.tensor.transpose(
                    tr[:, ct * 128:(ct + 1) * 128],
                    avn[:, ct * 128:(ct + 1) * 128],
                    ident[:])
            nc.scalar.activation(
                aT.rearrange("p (a c) -> p a c", a=4)[:, :, i * 128:(i + 1) * 128],
                tr.rearrange("p (a c) -> p a c", a=4),
                mybir.ActivationFunctionType.Copy)

            if KPH < 5:
                continue
            pr = pp_pr.tile([128, 512], F32, tag="pr")
            for ct in range(4):
                nc.tensor.matmul(
                    pr[:], aT[:, ct * NTOK + i * 128:ct * NTOK + (i + 1) * 128],
                    wp[ct][:], start=(ct == 0), stop=(ct == 3))
            bm = sb_o.tile([128, C], F32, tag="bm")
            nc.gpsimd.tensor_scalar_mul(bm[:], bpB[:], mq[:, i:i + 1])
            ot = sb_o.tile([128, C], F32, tag="ot")
            nc.vector.scalar_tensor_tensor(
                ot[:], pr[:], mq[:, i:i + 1], bm[:],
                op0=mybir.AluOpType.mult, op1=mybir.AluOpType.add)
            nc.sync.dma_start(outd[i * 128:(i + 1) * 128, :], ot[:])

    nc.compile()
    return nc


_CACHE: dict = {}


def _get_program() -> bacc.Bacc:
    if "nc" not in _CACHE:
        _CACHE["nc"] = build_program()
    return _CACHE["nc"]


def kernel(x, mask, Wq, bq, Wkv, bkv, Wproj, bproj) -> np.ndarray:
    x = np.asarray(x, np.float32)
    mask = np.asarray(mask, np.float32)
    band = np.ascontiguousarray(_band_mask())
    ident = np.ascontiguousarray(_identity())
    nc = _get_program()

    in_maps = []
    for core in range(8):
        b, h = divmod(core, 2)
        s = h * NTOK
        xc = np.zeros((NKV, C), np.float32)
        mc = np.zeros((NKV,), np.float32)
        lo, hi = max(0, s - PAD), min(T, s + NTOK + PAD)
        xc[lo - (s - PAD):lo - (s - PAD) + hi - lo] = x[b, lo:hi]
        mc[lo - (s - PAD):lo - (s - PAD) + hi - lo] = mask[b, lo:hi]
        in_maps.append({
            "x": xc, "mask": mc,
            "wq": np.asarray(Wq, np.float32), "bq": np.asarray(bq, np.float32),
            "wkv": np.asarray(Wkv, np.float32), "bkv": np.asarray(bkv, np.float32),
            "wproj": np.asarray(Wproj, np.float32),
            "bproj": np.asarray(bproj, np.float32),
            "band": band, "ident": ident,
        })

    res = bass_utils.run_bass_kernel_spmd(nc, in_maps, core_ids=list(range(8)))
    out = np.empty((B, T, C), np.float32)
    for core in range(8):
        b, h = divmod(core, 2)
        out[b, h * NTOK:(h + 1) * NTOK] = res.results[core]["out"]
    return out


# revision 14
# speedup vs baseline: 2.6729x; 1.0161x over previous
"""Local (sliding-window) self-attention Bass kernel for 8 TRN2 NeuronCores.

Problem: B=4, T=4096, C=512, H=8 heads, head_dim=64, window=15.
Sharding: 8 cores = batch(4) x seq-halves(2). Each core processes 2048 query
tokens of one batch element; its x chunk carries a 7-token halo on each side
(zero-padded at sequence edges, matching the reference's jnp.pad semantics),
padded to 2080 rows for 128/32 alignment.

v2 dataflow (no DMA transposes; scores computed k-major so alpha never needs
transposing; per-128-token-block batching of ACT/DVE work):
  x chunk --mask*cast--> x_tok bf16 --PE transpose--> xT (feature-major)
  qT/kT = W-stationary matmuls + bias (feature-major)
  v_tok = xT-stationary matmuls + bias, token-major, packed [64 v | 1 ones]
          per head (the ones column makes AV emit the softmax denominator)
  per 128-query block:
    scoresT [k,q] = kT-slice.T @ qT-slice per head (A:128k + B:16k chunks)
    one Exp over the whole block's scores -> alphaT bf16
    one DVE band-mask multiply
    AV token-major: av[q, 65] per head = alphaT.T @ v_aug  (den in col 64)
    reciprocal + one DVE normalize (rden broadcast via 0-stride AP) -> bf16
    4 PE transposes -> attnT feature-major -> one ACT copy -> aT
    proj: aT-stationary matmuls + bias + mask -> DMA out (token-major)
"""

import math
import os
from contextlib import ExitStack

import ml_dtypes
import numpy as np

import concourse.bacc as bacc
import concourse.bass as bass
import concourse.mybir as mybir
import concourse.tile as tile
from concourse import bass_utils

B, T, C, H, WIN = 4, 4096, 512, 8, 15
D = C // H            # 64
PAD = WIN // 2        # 7
NTOK = T // 2         # 2048 query tokens per core
NKV = 2112            # kv rows per core: 7 + 2048 + 7 = 2062, padded to 2112
                      # (the per-block k-tail chunk is 64 wide: 16*128+64)
NB = NTOK // 128      # 16 query blocks
SCALE = math.log(WIN) / D
F32 = mybir.dt.float32
BF16 = mybir.dt.bfloat16


M0 = 2000.0   # additive score mask; SCALE*M0 ~ 85 so exp underflows to 0


def _mask_consts() -> dict:
    """Additive band masks (transposed, as matmul lhsT) and a replicated
    identity used to broadcast them across the 4 head-columns of a bank.

    A-chunk in-band: k-14 <= q <= k.  B-chunk (k=128+r): q >= 114+r.
    """
    k = np.arange(128)[:, None]
    q = np.arange(128)[None, :]
    a = np.where((q >= k - 14) & (q <= k), 0.0, -M0).astype(np.float32)
    r = np.arange(64)[:, None]
    b = np.where(q >= 114 + r, 0.0, -M0).astype(np.float32)
    identj = np.zeros((128, 512), np.float32)
    for j in range(4):
        identj[:, j * 128:(j + 1) * 128] = np.eye(128)
    return {
        "bandat": np.ascontiguousarray(a.T.astype(ml_dtypes.bfloat16)),
        "bandbt": np.ascontiguousarray(b.T.astype(ml_dtypes.bfloat16)),
        "identj": np.ascontiguousarray(identj.astype(ml_dtypes.bfloat16)),
    }


def _identity() -> np.ndarray:
    return np.eye(128, dtype=ml_dtypes.bfloat16)


def build_program() -> bacc.Bacc:
    nc = bacc.Bacc("TRN2", target_bir_lowering=False, debug=False,
                   enable_asserts=False, num_devices=8)

    xd = nc.dram_tensor("x", [NKV, C], F32, kind="ExternalInput").ap()
    maskd = nc.dram_tensor("mask", [NKV], F32, kind="ExternalInput").ap()
    wqd = nc.dram_tensor("wq", [C, C], F32, kind="ExternalInput").ap()
    bqd = nc.dram_tensor("bq", [C], F32, kind="ExternalInput").ap()
    wkvd = nc.dram_tensor("wkv", [C, 2 * C], F32, kind="ExternalInput").ap()
    bkvd = nc.dram_tensor("bkv", [2 * C], F32, kind="ExternalInput").ap()
    wpd = nc.dram_tensor("wproj", [C, C], F32, kind="ExternalInput").ap()
    bpd = nc.dram_tensor("bproj", [C], F32, kind="ExternalInput").ap()
    bandatd = nc.dram_tensor("bandat", [128, 128], BF16, kind="ExternalInput").ap()
    bandbtd = nc.dram_tensor("bandbt", [128, 64], BF16, kind="ExternalInput").ap()
    identjd = nc.dram_tensor("identj", [128, 512], BF16, kind="ExternalInput").ap()
    identd = nc.dram_tensor("ident", [128, 128], BF16, kind="ExternalInput").ap()
    outd = nc.dram_tensor("out", [NTOK, C], F32, kind="ExternalOutput").ap()

    with tile.TileContext(nc) as tc, ExitStack() as ctx:
        sb = ctx.enter_context(tc.tile_pool(name="sb", bufs=1))
        sb_ld = ctx.enter_context(tc.tile_pool(name="sb_ld", bufs=3))
        sb_a = ctx.enter_context(tc.tile_pool(name="sb_a", bufs=3))
        sb_o = ctx.enter_context(tc.tile_pool(name="sb_o", bufs=3))
        pp_sc = ctx.enter_context(tc.tile_pool(name="pp_sc", bufs=1, space="PSUM"))
        pp_tr = ctx.enter_context(tc.tile_pool(name="pp_tr", bufs=1, space="PSUM"))
        pp_pr = ctx.enter_context(tc.tile_pool(name="pp_pr", bufs=1, space="PSUM"))
        pp_av = ctx.enter_context(tc.tile_pool(name="pp_av", bufs=1, space="PSUM"))

        # ---- persistent SBUF tensors ----
        xT = sb.tile([128, 4 * NKV], BF16, tag="xT")     # col ci*NKV + t
        qT = sb.tile([128, 4 * NTOK], BF16, tag="qT")    # col co*NTOK + t
        kT = sb.tile([128, 4 * NKV], BF16, tag="kT")     # col co*NKV + t
        aT = sb.tile([128, 4 * NTOK], BF16, tag="aT")    # col ct*NTOK + q
        v_tok = [sb.tile([128, 520], BF16, tag=f"vtok{i}", name=f"vtok{i}")
                 for i in range(17)]                     # col h*65: [64 v | 1]
        bandat = sb.tile([128, 128], BF16, tag="bandat")
        bandbt = sb.tile([128, 64], BF16, tag="bandbt")
        identj = sb.tile([128, 512], BF16, tag="identj")
        ident = sb.tile([128, 128], BF16, tag="ident")
        mqr = sb.tile([1, NTOK], BF16, tag="mqr")     # mask row (proj bias fold)
        bpr = sb.tile([1, C], BF16, tag="bpr")        # bproj row
        wq = [sb.tile([128, C], BF16, tag=f"wq{i}", name=f"wq{i}") for i in range(4)]
        wk = [sb.tile([128, C], BF16, tag=f"wk{i}", name=f"wk{i}") for i in range(4)]
        wv = [sb.tile([128, C], BF16, tag=f"wv{i}", name=f"wv{i}") for i in range(4)]
        wp = [sb.tile([128, C], BF16, tag=f"wp{i}", name=f"wp{i}") for i in range(4)]
        bq_t = sb.tile([128, 4], F32, tag="bq")       # per-partition q bias
        bk_t = sb.tile([128, 4], F32, tag="bk")       # per-partition k bias
        bvB = sb.tile([128, C], F32, tag="bvB")       # v bias bcast over partitions
        mq = sb.tile([128, NB], F32, tag="mq")        # query-token mask, per block

        # ---- constants / weights in ----
        nc.sync.dma_start(bandat[:], bandatd)
        nc.sync.dma_start(bandbt[:], bandbtd)
        nc.sync.dma_start(identj[:], identjd)
        nc.sync.dma_start(ident[:], identd)
        mqf = sb_ld.tile([1, NTOK], F32, tag="mqf")
        nc.sync.dma_start(mqf[:], maskd[PAD:PAD + NTOK][None, :])
        nc.vector.tensor_copy(mqr[:], mqf[:])
        bprf = sb_ld.tile([1, C], F32, tag="bprf")
        nc.sync.dma_start(bprf[:], bpd[None, :])
        nc.vector.tensor_copy(bpr[:], bprf[:])
        nc.sync.dma_start(bq_t[:], bqd.rearrange("(a b) -> b a", b=128))
        nc.sync.dma_start(bk_t[:], bkvd[0:C].rearrange("(a b) -> b a", b=128))
        nc.sync.dma_start(bvB[:], bkvd[C:2 * C][None, :].broadcast_to((128, C)))
        nc.sync.dma_start(mq[:], maskd[PAD:PAD + NTOK].rearrange("(a b) -> b a", b=128))
        for ci in range(4):
            wqf = sb_ld.tile([128, C], F32, tag="wld")
            nc.sync.dma_start(wqf[:], wqd[ci * 128:(ci + 1) * 128, :])
            nc.vector.tensor_copy(wq[ci][:], wqf[:])
            wkf = sb_ld.tile([128, 2 * C], F32, tag="wld2")
            nc.sync.dma_start(wkf[:], wkvd[ci * 128:(ci + 1) * 128, :])
            nc.vector.tensor_copy(wk[ci][:], wkf[:, 0:C])
            nc.vector.tensor_copy(wv[ci][:], wkf[:, C:2 * C])
            wpf = sb_ld.tile([128, C], F32, tag="wld")
            nc.sync.dma_start(wpf[:], wpd[ci * 128:(ci + 1) * 128, :])
            nc.vector.tensor_copy(wp[ci][:], wpf[:])

        # ---- x in: mask*cast, then PE-transpose to feature-major ----
        for t in range(17):
            r0, r1 = t * 128, min((t + 1) * 128, NKV)
            rows = r1 - r0
            xf = sb_ld.tile([128, C], F32, tag="xf")
            nc.sync.dma_start(xf[:rows, :], xd[r0:r1, :])
            mrow = sb_ld.tile([128, 1], F32, tag="mrow")
            nc.sync.dma_start(mrow[:rows, :], maskd[r0:r1][:, None])
            xb = sb_ld.tile([128, C], BF16, tag="xb")
            nc.vector.tensor_scalar_mul(xb[:rows, :], xf[:rows, :], mrow[:rows, :])
            xtr = pp_tr.tile([128, 512], BF16, tag="tr", name="xtr")
            for ci in range(4):
                nc.tensor.transpose(
                    xtr[:, ci * 128:ci * 128 + rows],
                    xb[:rows, ci * 128:(ci + 1) * 128],
                    ident[:rows, :rows])
            nc.scalar.activation(
                xT.rearrange("p (a c) -> p a c", a=4)[:, :, r0:r1],
                xtr.rearrange("p (a c) -> p a c", a=4)[:, :, 0:rows],
                mybir.ActivationFunctionType.Copy)

        # Alternate projection-phase PSUM tiles between the two big pools so
        # consecutive chunks double-buffer (each pool alone has bufs=1).
        pcnt = [0]

        def proj_ps():
            pool = pp_sc if pcnt[0] % 2 == 0 else pp_av
            pcnt[0] += 1
            shape = [128, 2048] if pool is pp_sc else [128, 1024]
            return pool.tile(shape, F32, tag="sc" if pool is pp_sc else "av",
                             name=f"pps{pcnt[0]}")

        # ---- qT (feature-major): W stationary, xT moving ----
        _P1 = 4 if int(os.environ.get("KPH", "5")) >= 1 else 0
        for co in range(_P1):
            for ch in range(4):
                t0 = ch * 512
                ps = proj_ps()
                for ci in range(4):
                    nc.tensor.matmul(
                        ps[:, 0:512], wq[ci][:, co * 128:(co + 1) * 128],
                        xT[:, ci * NKV + PAD + t0:ci * NKV + PAD + t0 + 512],
                        start=(ci == 0), stop=(ci == 3))
                nc.scalar.activation(qT[:, co * NTOK + t0:co * NTOK + t0 + 512],
                                     ps[:, 0:512],
                                     mybir.ActivationFunctionType.Identity,
                                     bias=bq_t[:, co:co + 1])

        # ---- kT (feature-major) ----
        KCH = [512, 512, 512, 512, 64]
        for co in range(_P1):
            t0 = 0
            for w in KCH:
                ps = proj_ps()
                for ci in range(4):
                    nc.tensor.matmul(
                        ps[:, 0:w], wk[ci][:, co * 128:(co + 1) * 128],
                        xT[:, ci * NKV + t0:ci * NKV + t0 + w],
                        start=(ci == 0), stop=(ci == 3))
                nc.scalar.activation(kT[:, co * NKV + t0:co * NKV + t0 + w],
                                     ps[:, 0:w],
                                     mybir.ActivationFunctionType.Identity,
                                     bias=bk_t[:, co:co + 1])
                t0 += w

        # ---- v_tok (token-major, packed [64 v | ones] per head) ----
        for t in range(17 if _P1 else 0):
            r0, r1 = t * 128, min((t + 1) * 128, NKV)
            rows = r1 - r0
            ps = proj_ps()
            for ci in range(4):
                nc.tensor.matmul(
                    ps[:rows, 0:512], xT[:, ci * NKV + r0:ci * NKV + r1],
                    wv[ci][:], start=(ci == 0), stop=(ci == 3))
            vv = v_tok[t].rearrange("p (h y) -> p h y", h=8)
            nc.gpsimd.memset(vv[:, :, 64:65], 1.0)
            nc.vector.scalar_tensor_tensor(
                vv[:rows, :, 0:64],
                ps[:rows, 0:512].rearrange("p (h y) -> p h y", h=8),
                1.0,
                bvB.rearrange("p (h y) -> p h y", h=8)[:rows],
                op0=mybir.AluOpType.mult, op1=mybir.AluOpType.add)

        # ---- attention: per 128-query block ----
        KPH = int(os.environ.get("KPH", "5"))
        for i in range(NB if KPH >= 2 else 0):
            sc = pp_sc.tile([128, 2048], F32, tag="sc")
            # Heads grouped by operand partition base per PSUM bank (a
            # matmul's tile_position row must be uniform within a bank):
            # even heads (base 0) fill bank 0/2, odd heads (base 64) 1/3.
            # Each bank is one accumulation group: 4 head scores writing
            # disjoint quarters, then one additive band-mask matmul
            # (band @ [I I I I]) over the whole bank; out-of-band scores
            # drop to ~-2000 so exp underflows to exactly 0.
            for b in range(2):
                for j in range(4):
                    h = 2 * j + b
                    co, hr = h // 2, (h % 2) * 64
                    nc.tensor.matmul(
                        sc[:, b * 512 + j * 128:b * 512 + (j + 1) * 128],
                        kT[hr:hr + 64, co * NKV + i * 128:co * NKV + i * 128 + 128],
                        qT[hr:hr + 64, co * NTOK + i * 128:co * NTOK + (i + 1) * 128],
                        start=(j == 0), stop=False, skip_group_check=True)
                nc.tensor.matmul(
                    sc[:, b * 512:(b + 1) * 512], bandat[:], identj[:],
                    start=False, stop=True, skip_group_check=True)
            for b in range(2):
                for j in range(4):
                    h = 2 * j + b
                    co, hr = h // 2, (h % 2) * 64
                    c0 = 1024 + b * 512 + j * 128
                    nc.tensor.matmul(
                        sc[0:64, c0:c0 + 128],
                        kT[hr:hr + 64, co * NKV + i * 128 + 128:co * NKV + i * 128 + 192],
                        qT[hr:hr + 64, co * NTOK + i * 128:co * NTOK + (i + 1) * 128],
                        start=(j == 0), stop=False, skip_group_check=True)
                nc.tensor.matmul(
                    sc[0:64, 1024 + b * 512:1024 + (b + 1) * 512],
                    bandbt[:], identj[:],
                    start=False, stop=True, skip_group_check=True)
            alpha = sb_a.tile([128, 2048], BF16, tag="alpha")
            nc.scalar.activation(alpha[:, 0:1024], sc[:, 0:1024],
                                 mybir.ActivationFunctionType.Exp, scale=SCALE)
            nc.scalar.activation(alpha[0:64, 1024:2048], sc[0:64, 1024:2048],
                                 mybir.ActivationFunctionType.Exp, scale=SCALE)
            if KPH < 3:
                continue

            av = pp_av.tile([128, 1024], F32, tag="av")
            for h in range(8):
                c0 = (h // 4) * 512 + (h % 4) * 65
                ac = (h % 2) * 512 + (h // 2) * 128
                bc = 1024 + ac
                nc.tensor.matmul(
                    av[:, c0:c0 + 65],
                    alpha[:, ac:ac + 128],
                    v_tok[i][:, h * 65:h * 65 + 65],
                    start=True, stop=False)
                nc.tensor.matmul(
                    av[:, c0:c0 + 65],
                    alpha[0:64, bc:bc + 128],
                    v_tok[i + 1][0:64, h * 65:h * 65 + 65],
                    start=False, stop=True)
            avv = (av.rearrange("p (a c) -> p a c", a=2)[:, :, 0:260]
                     .rearrange("p a (h y) -> p a h y", h=4))
            rden = sb_o.tile([128, 8], F32, tag="rden")
            nc.vector.reciprocal(rden.rearrange("p (a h) -> p a h", a=2),
                                 avv[:, :, :, 64:65].squeeze(3))
            nc.vector.tensor_scalar_mul(rden[:], rden[:], mq[:, i:i + 1])
            avn = sb_o.tile([128, 512], BF16, tag="avn")
            for a in range(2):
                nc.vector.scalar_tensor_tensor(
                    avn[:, a * 256:(a + 1) * 256]
                       .rearrange("p (h y) -> p h y", h=4),
                    avv[:, a:a + 1, :, 0:64].squeeze(1), 1.0,
                    rden[:, a * 4:(a + 1) * 4].unsqueeze(2)
                        .broadcast_to((128, 4, 64)),
                    op0=mybir.AluOpType.mult, op1=mybir.AluOpType.mult)

            if KPH < 4:
                continue
            tr = pp_tr.tile([128, 512], BF16, tag="tr")
            for ct in range(4):
                nc.tensor.transpose(
                    tr[:, ct * 128:(ct + 1) * 128],
                    avn[:, ct * 128:(ct + 1) * 128],
                    ident[:])
            nc.scalar.activation(
                aT.rearrange("p (a c) -> p a c", a=4)[:, :, i * 128:(i + 1) * 128],
                tr.rearrange("p (a c) -> p a c", a=4),
                mybir.ActivationFunctionType.Copy)

            if KPH < 5:
                continue
            pr = pp_pr.tile([128, 512], F32, tag="pr")
            for ct in range(4):
                nc.tensor.matmul(
                    pr[:], aT[:, ct * NTOK + i * 128:ct * NTOK + (i + 1) * 128],
                    wp[ct][:], start=(ct == 0), stop=False,
                    skip_group_check=True)
            nc.tensor.matmul(
                pr[:], mqr[0:1, i * 128:(i + 1) * 128], bpr[:],
                start=False, stop=True, skip_group_check=True)
            ot = sb_o.tile([128, C], F32, tag="ot")
            nc.vector.tensor_copy(ot[:], pr[:])
            nc.sync.dma_start(outd[i * 128:(i + 1) * 128, :], ot[:])

    nc.compile()
    return nc


_CACHE: dict = {}


def _get_program() -> bacc.Bacc:
    if "nc" not in _CACHE:
        _CACHE["nc"] = build_program()
    return _CACHE["nc"]


def kernel(x, mask, Wq, bq, Wkv, bkv, Wproj, bproj) -> np.ndarray:
    x = np.asarray(x, np.float32)
    mask = np.asarray(mask, np.float32)
    consts = _mask_consts()
    ident = np.ascontiguousarray(_identity())
    nc = _get_program()

    in_maps = []
    for core in range(8):
        b, h = divmod(core, 2)
        s = h * NTOK
        xc = np.zeros((NKV, C), np.float32)
        mc = np.zeros((NKV,), np.float32)
        lo, hi = max(0, s - PAD), min(T, s + NTOK + PAD)
        xc[lo - (s - PAD):lo - (s - PAD) + hi - lo] = x[b, lo:hi]
        mc[lo - (s - PAD):lo - (s - PAD) + hi - lo] = mask[b, lo:hi]
        in_maps.append({
            "x": xc, "mask": mc,
            "wq": np.asarray(Wq, np.float32), "bq": np.asarray(bq, np.float32),
            "wkv": np.asarray(Wkv, np.float32), "bkv": np.asarray(bkv, np.float32),
            "wproj": np.asarray(Wproj, np.float32),
            "bproj": np.asarray(bproj, np.float32),
            "ident": ident, **consts,
        })

    res = bass_utils.run_bass_kernel_spmd(nc, in_maps, core_ids=list(range(8)))
    out = np.empty((B, T, C), np.float32)
    for core in range(8):
        b, h = divmod(core, 2)
        out[b, h * NTOK:(h + 1) * NTOK] = res.results[core]["out"]
    return out
